# revision 1
# baseline (speedup 1.0000x reference)
"""Trainium2 Bass kernel for nn_CrossAttentionBlock (LN -> MHA -> out-proj -> residual).

Sharding: 8 cores = 2 batches x 4 head-groups (2 heads each). Each core:
  - computes LN stats of its batch's x ([c, seq] layout, stats via ones-matmul),
  - projects Q/K/V for its 2 heads (gamma/beta folded into weights on host),
  - flash-style attention: exp on ACT (bf16 probs), AV+sumexp via [1|V] matmul,
  - partial out-proj with its Wo rows.
Host sums the 4 partials per batch and adds bias + residual.
"""
import numpy as np

C = 512
SEQ = 2048
P = 128
NCH = 4          # c chunks of 128
DH = 64
HPC = 2          # heads per core
EPS = 1e-5

_CACHE = {}
_LAST_IN_MAPS = None


def _build():
    import concourse.bass as bass
    import concourse.tile as tile
    from concourse import bacc, mybir
    from concourse.masks import make_identity

    F32 = mybir.dt.float32
    F32R = mybir.dt.float32r
    BF16 = mybir.dt.bfloat16
    AF = mybir.ActivationFunctionType
    ALU = mybir.AluOpType

    nc = bacc.Bacc("TRN2", target_bir_lowering=False, debug=False,
                   enable_asserts=False, num_devices=8)

    xb_d = nc.dram_tensor("xb", [C, SEQ], F32, kind="ExternalInput").ap()
    aq_d = nc.dram_tensor("aq", [C, P], F32, kind="ExternalInput").ap()
    ak_d = nc.dram_tensor("ak", [C, P], F32, kind="ExternalInput").ap()
    av_d = nc.dram_tensor("av", [C, P], F32, kind="ExternalInput").ap()
    wo_d = nc.dram_tensor("wo", [P, C], F32, kind="ExternalInput").ap()
    uq_d = nc.dram_tensor("uq", [1, P], F32, kind="ExternalInput").ap()
    uk_d = nc.dram_tensor("uk", [1, P], F32, kind="ExternalInput").ap()
    uv_d = nc.dram_tensor("uv", [1, P], F32, kind="ExternalInput").ap()
    vq_d = nc.dram_tensor("vq", [P, 1], F32, kind="ExternalInput").ap()
    vk_d = nc.dram_tensor("vk", [P, 1], F32, kind="ExternalInput").ap()
    yp_d = nc.dram_tensor("yp", [C, SEQ], F32, kind="ExternalOutput").ap()

    with tile.TileContext(nc) as tc:
        with tc.tile_pool(name="sb", bufs=1) as sb, \
             tc.tile_pool(name="ep", bufs=4) as ep, \
             tc.tile_pool(name="pa", bufs=1, space="PSUM") as pa, \
             tc.tile_pool(name="pb", bufs=1, space="PSUM") as pb:

            # ---- constant / weight loads ----
            aw = {}
            for name, d in (("aq", aq_d), ("ak", ak_d), ("av", av_d)):
                t = sb.tile([P, NCH, P], F32R, tag=name)
                nc.sync.dma_start(
                    t[:], d.rearrange("(k p) m -> p k m", p=P).bitcast(F32R))
                aw[name] = t
            wo_t = sb.tile([P, C], F32R, tag="wo")
            nc.sync.dma_start(wo_t[:], wo_d[:, :].bitcast(F32R))
            uvec = {}
            for name, d in (("uq", uq_d), ("uk", uk_d), ("uv", uv_d)):
                t = sb.tile([1, P], F32R, tag=name)
                nc.sync.dma_start(t[:], d[:, :].bitcast(F32R))
                uvec[name] = t
            vq_t = sb.tile([P, 1], F32, tag="vq")
            nc.sync.dma_start(vq_t[:], vq_d[:, :])
            vk_t = sb.tile([P, 1], F32, tag="vk")
            nc.sync.dma_start(vk_t[:], vk_d[:, :])
            ones_f = sb.tile([P, 2], F32, tag="onesf")
            nc.vector.memset(ones_f[:], 1.0 / C)
            ones_t = sb.tile([P, 2], F32R, tag="ones")
            nc.vector.tensor_copy(ones_t[:], ones_f[:])
            eps_t = sb.tile([1, 1], F32, tag="eps")
            nc.vector.memset(eps_t[:], EPS)
            ident_f = sb.tile([P, P], F32, tag="idf")
            make_identity(nc, ident_f[:])
            ident = sb.tile([P, P], F32R, tag="id")
            nc.vector.tensor_copy(ident[:], ident_f[:])

            # ---- x load + square ----
            xt = []
            for k in range(NCH):
                t = sb.tile([P, SEQ], F32R, tag=f"x{k}")
                nc.sync.dma_start(t[:], xb_d[k * P:(k + 1) * P, :].bitcast(F32R))
                xt.append(t)
            xsq = []
            for k in range(NCH):
                t = sb.tile([P, SEQ], F32R, tag=f"q{k}")
                nc.vector.tensor_tensor(t[:], xt[k][:], xt[k][:], ALU.mult)
                xsq.append(t)

            # ---- LN stats: s1 then s2 through the same psum tile ----
            st_ps = pb.tile([2, NCH, 512], F32, tag="b")
            for nb in range(NCH):
                for k in range(NCH):
                    nc.tensor.matmul(st_ps[0:2, nb, :], ones_t[:],
                                     xt[k][:, nb * 512:(nb + 1) * 512],
                                     start=(k == 0), stop=(k == 3))
            mu_sb = sb.tile([1, SEQ], F32, tag="musb")
            nc.vector.tensor_copy(mu_sb[:], st_ps[0:1, :, :])
            st2_ps = pb.tile([2, NCH, 512], F32, tag="b")
            for nb in range(NCH):
                for k in range(NCH):
                    nc.tensor.matmul(st2_ps[0:2, nb, :], ones_t[:],
                                     xsq[k][:, nb * 512:(nb + 1) * 512],
                                     start=(k == 0), stop=(k == 3))
            mu = mu_sb[0:1, :]
            e2 = st2_ps[0:1, :, :]
            musq = sb.tile([1, SEQ], F32, tag="musq")
            nc.vector.tensor_tensor(musq[:], mu, mu, ALU.mult)
            varr = sb.tile([1, SEQ], F32, tag="varr")
            nc.vector.tensor_tensor(varr[:], e2, musq[:], ALU.subtract)
            lnv = sb.tile([1, SEQ], F32, tag="lnv")
            nc.scalar.activation(lnv[:], varr[:], AF.Ln, bias=eps_t[0:1, :], scale=1.0)
            rs_row = sb.tile([1, SEQ], F32, tag="rsr")
            nc.scalar.activation(rs_row[:], lnv[:], AF.Exp, bias=0.0, scale=-0.5)
            m2_row = sb.tile([1, SEQ], F32R, tag="m2r")
            nc.vector.tensor_tensor(m2_row[:], mu, rs_row[:], ALU.mult)
            rs_b = sb.tile([P, SEQ], F32, tag="rsb")
            nc.gpsimd.partition_broadcast(rs_b[:], rs_row[:], channels=P)

            # ---- z = x * rs (per-token scale; mean-shift folded via rank-1) ----
            z = []
            for k in range(NCH):
                t = sb.tile([P, SEQ], F32R, tag=f"q{k}")
                nc.vector.tensor_tensor(t[:], xt[k][:], rs_b[:], ALU.mult)
                z.append(t)

            # ---- projections ----
            def project(w_t, u_t, out_sb, bias_t):
                for nb in range(4):
                    slot = pa.tile([P, 512], F32, tag=f"s{nb % 2}")
                    for k in range(NCH):
                        nc.tensor.matmul(slot[:], w_t[:, k, :],
                                         z[k][:, nb * 512:(nb + 1) * 512],
                                         start=(k == 0), stop=False)
                    nc.tensor.matmul(slot[:], u_t[:],
                                     m2_row[0:1, nb * 512:(nb + 1) * 512],
                                     start=False, stop=True)
                    if bias_t is not None:
                        nc.vector.tensor_scalar(
                            out=out_sb[:, nb * 512:(nb + 1) * 512], in0=slot[:],
                            scalar1=bias_t[:, 0:1], scalar2=None, op0=ALU.add)
                    else:
                        nc.vector.tensor_copy(
                            out_sb[:, nb * 512:(nb + 1) * 512], slot[:])

            qt_sb = sb.tile([P, SEQ], F32R, tag="qt")
            project(aw["aq"], uvec["uq"], qt_sb, vq_t)
            kt_sb = sb.tile([P, SEQ], F32R, tag="kt")
            project(aw["ak"], uvec["uk"], kt_sb, vk_t)
            vt_sb = sb.tile([P, SEQ], F32R, tag="vt")
            project(aw["av"], uvec["uv"], vt_sb, None)

            # ---- V -> [j, d] layout via PE transpose; bf16 [1|0*63|V64] per head ----
            v_sb = sb.tile([P, 16, 256], BF16, tag="vsb")
            nc.vector.memset(v_sb[:], 0.0)
            nc.vector.memset(
                v_sb[:].rearrange("p j (h c) -> p j h c", c=128)[:, :, :, 0:1], 1.0)
            for jb in range(16):
                tr = pa.tile([P, P], F32R, tag=f"s{jb % 2}")
                nc.tensor.transpose(tr[:], vt_sb[:, jb * P:(jb + 1) * P], ident[:])
                nc.vector.tensor_copy(
                    v_sb[:, jb, :].rearrange("p (h c) -> p h c", c=128)[:, :, 64:128],
                    tr[:].rearrange("p (h c) -> p h c", c=64))

            # ---- attention ----
            attn_sb = sb.tile([P, SEQ], F32R, tag="at")
            for ig in range(2):
                i0 = ig * 1024
                av_ps = pb.tile([P, HPC, 1024], F32, tag="b")
                for jb in range(16):
                    sts = []
                    for h in range(HPC):
                        st = pa.tile([P, 1024], F32, tag=f"s{h}")
                        for nb in range(2):
                            nc.tensor.matmul(
                                st[:, nb * 512:(nb + 1) * 512],
                                kt_sb[h * DH:(h + 1) * DH, jb * P:(jb + 1) * P],
                                qt_sb[h * DH:(h + 1) * DH,
                                      i0 + nb * 512:i0 + (nb + 1) * 512],
                                start=True, stop=True,
                                tile_position=(h * DH, 0))
                        sts.append(st)
                    for h in range(HPC):
                        e_t = ep.tile([P, 1024], BF16, tag="e")
                        nc.scalar.activation(e_t[:], sts[h][:], AF.Exp,
                                             bias=0.0, scale=1.0)
                        for nb in range(2):
                            nc.tensor.matmul(
                                av_ps[:, h, nb * 512:(nb + 1) * 512],
                                v_sb[:, jb, 128 * h:128 * h + 128],
                                e_t[:, nb * 512:(nb + 1) * 512],
                                start=(jb == 0), stop=(jb == 15))
                # normalize: row 0 of av is sumexp
                for h in range(HPC):
                    se = sb.tile([1, 1024], F32, tag="se")
                    nc.vector.tensor_copy(se[:], av_ps[0:1, h, :])
                    nc.vector.reciprocal(se[:], se[:])
                    rb = sb.tile([P, 1024], F32, tag="rb")
                    nc.gpsimd.partition_broadcast(rb[:], se[:], channels=P)
                    nc.vector.tensor_tensor(
                        attn_sb[h * DH:(h + 1) * DH, i0:i0 + 1024],
                        av_ps[64:128, h, :], rb[64:128, :], ALU.mult)

            # ---- out-proj partial: yp = wo.T @ attn ----
            for m in range(4):
                yp_sb = sb.tile([P, SEQ], F32, tag=f"x{m}")
                for nb in range(4):
                    slot = pa.tile([P, 512], F32, tag=f"s{nb % 2}")
                    nc.tensor.matmul(slot[:], wo_t[:, m * P:(m + 1) * P],
                                     attn_sb[:, nb * 512:(nb + 1) * 512],
                                     start=True, stop=True)
                    nc.vector.tensor_copy(yp_sb[:, nb * 512:(nb + 1) * 512], slot[:])
                nc.sync.dma_start(yp_d[m * P:(m + 1) * P, :], yp_sb[:])

    nc.compile()
    return nc


def kernel(x, Wq, Wk, Wv, Wo, bo, gamma, beta):
    from concourse import bass_utils

    x = np.asarray(x, np.float32)
    Wq, Wk, Wv, Wo = (np.asarray(w, np.float32) for w in (Wq, Wk, Wv, Wo))
    bo, gamma, beta = (np.asarray(v, np.float32) for v in (bo, gamma, beta))
    b = x.shape[0]
    xs = x.reshape(b, C, SEQ)

    s = DH ** -0.5
    aq_f = gamma[:, None] * Wq * s
    ak_f = gamma[:, None] * Wk
    av_f = gamma[:, None] * Wv
    vq_f = (Wq.T @ beta) * s
    vk_f = Wk.T @ beta
    vv_f = Wv.T @ beta

    if "nc" not in _CACHE:
        _CACHE["nc"] = _build()
    nc = _CACHE["nc"]

    in_maps = []
    for core in range(8):
        bi, hg = divmod(core, 4)
        cs = slice(hg * P, (hg + 1) * P)
        in_maps.append({
            "xb": np.ascontiguousarray(xs[bi]),
            "aq": np.ascontiguousarray(aq_f[:, cs]),
            "ak": np.ascontiguousarray(ak_f[:, cs]),
            "av": np.ascontiguousarray(av_f[:, cs]),
            "wo": np.ascontiguousarray(Wo[cs, :]),
            "uq": -aq_f[:, cs].sum(0)[None, :].astype(np.float32),
            "uk": -ak_f[:, cs].sum(0)[None, :].astype(np.float32),
            "uv": -av_f[:, cs].sum(0)[None, :].astype(np.float32),
            "vq": vq_f[cs, None].astype(np.float32),
            "vk": vk_f[cs, None].astype(np.float32),
        })

    global _LAST_IN_MAPS
    _LAST_IN_MAPS = in_maps
    res = bass_utils.run_bass_kernel_spmd(nc, in_maps, core_ids=list(range(8)))
    bias_total = bo + Wo.T @ vv_f
    y = np.empty((b, C, SEQ), np.float32)
    for bi in range(b):
        acc = xs[bi] + bias_total[:, None]
        for hg in range(4):
            acc = acc + res.results[bi * 4 + hg]["yp"]
        y[bi] = acc
    return y.reshape(x.shape).astype(np.float32)



# revision 10
# speedup vs baseline: 1.6707x; 1.6707x over previous
"""Trainium2 Bass kernel for nn_CrossAttentionBlock (LN -> MHA -> out-proj -> residual).

Sharding: 8 cores = 2 batches x 4 head-groups (2 heads each). Each core:
  - streams raw x (bf16), computes LN stats via ones-matmul while projecting
    Q/K/V on raw x; the LN scale rs and mean/bias corrections are folded into
    the PSUM drain (elementwise *rs) and a rank-2 fix matmul,
  - software-pipelined attention: QK -> exp(ACT) -> AV with double-buffered
    score banks so PE and ACT both stay busy,
  - sumexp via ones-column embedded in the V weights; per-head normalize
    overlaps the other head's attention,
  - out-proj partial written as bf16; host sums the 4 partials per batch and
    adds bias + residual.
"""
import numpy as np

C = 512
SEQ = 2048
P = 128
NCH = 4          # c chunks of 128
DH = 64
HPC = 2          # heads per core
IB = 1024        # attention i-block
EPS = 1e-5

_CACHE = {}
_LAST_IN_MAPS = None


def _build():
    import concourse.bass as bass
    import concourse.tile as tile
    from concourse import bacc, mybir
    from concourse.masks import make_identity

    F32 = mybir.dt.float32
    BF16 = mybir.dt.bfloat16
    AF = mybir.ActivationFunctionType
    ALU = mybir.AluOpType

    nc = bacc.Bacc("TRN2", target_bir_lowering=False, debug=False,
                   enable_asserts=False, num_devices=8)

    xb_d = nc.dram_tensor("xb", [C, SEQ], BF16, kind="ExternalInput").ap()
    aq_d = nc.dram_tensor("aq", [P, C], BF16, kind="ExternalInput").ap()
    ak_d = nc.dram_tensor("ak", [P, C], BF16, kind="ExternalInput").ap()
    av_d = nc.dram_tensor("av", [P, C], BF16, kind="ExternalInput").ap()
    wo_d = nc.dram_tensor("wo", [P, C], BF16, kind="ExternalInput").ap()
    # rank-1 fix weight rows packed on one partition: [qu, qv, ku, kv, vu]
    f_d = nc.dram_tensor("f", [1, 5 * P], BF16, kind="ExternalInput").ap()
    yp_d = nc.dram_tensor("yp", [C, SEQ], BF16, kind="ExternalOutput").ap()

    with tile.TileContext(nc) as tc:
        with tc.tile_pool(name="sb", bufs=1) as sb, \
             tc.tile_pool(name="ep", bufs=1) as ep, \
             tc.tile_pool(name="pa", bufs=1, space="PSUM") as pa:

            # ---- input DMAs (x first; stats start as chunks land) ----
            xt = []
            for k in range(NCH):
                t = sb.tile([P, SEQ], BF16, tag=f"x{k}")
                nc.sync.dma_start(t[:], xb_d[k * P:(k + 1) * P, :])
                xt.append(t)
            aw = {}
            for name, d in (("aq", aq_d), ("ak", ak_d), ("av", av_d)):
                t = sb.tile([P, NCH, P], BF16, tag=name)
                nc.sync.dma_start(t[:], d.rearrange("p (k m) -> p k m", k=NCH))
                aw[name] = t
            wo_t = sb.tile([P, C], BF16, tag="wo")
            nc.sync.dma_start(wo_t[:], wo_d[:, :])
            f_t = sb.tile([1, 5, P], BF16, tag="f")
            nc.sync.dma_start(f_t[:], f_d.rearrange("o (i p) -> o i p", i=5))

            # ---- constants ----
            ones_t = sb.tile([P, 2], BF16, tag="ones")
            nc.vector.memset(ones_t[:], 1.0 / C)
            eps_t = sb.tile([1, 1], F32, tag="eps")
            nc.vector.memset(eps_t[:], EPS)
            ident_f = sb.tile([P, P], F32, tag="idf")
            make_identity(nc, ident_f[:])
            ident = sb.tile([P, P], BF16, tag="id")
            nc.vector.tensor_copy(ident[:], ident_f[:])
            v_sb = sb.tile([P, 16, 256], BF16, tag="vsb")
            nc.vector.memset(v_sb[:], 0.0)
            nc.vector.memset(
                v_sb[:].rearrange("p j (h c) -> p j h c", c=128)[:, :, :, 0:1], 1.0)

            # ---- squares (DVE, per chunk as x lands) ----
            xq = []
            for k in range(NCH):
                t = sb.tile([P, SEQ], BF16, tag=f"q{k}")
                nc.vector.tensor_tensor(t[:], xt[k][:], xt[k][:], ALU.mult)
                xq.append(t)

            # ---- LN stats: s1 (mean) in sc0/sc1, s2 (E[x^2]) in av ----
            s1a = pa.tile([2, IB], F32, tag="sc0")
            s1b = pa.tile([2, IB], F32, tag="sc1")
            for k in range(NCH):
                for nb in range(4):
                    dst = s1a if nb < 2 else s1b
                    nc.tensor.matmul(dst[:, (nb % 2) * 512:(nb % 2 + 1) * 512],
                                     ones_t[:],
                                     xt[k][:, nb * 512:(nb + 1) * 512],
                                     start=(k == 0), stop=(k == NCH - 1))
            s2 = pa.tile([2, SEQ], F32, tag="av")
            for k in range(NCH):
                for nb in range(4):
                    nc.tensor.matmul(s2[:, nb * 512:(nb + 1) * 512],
                                     ones_t[:],
                                     xq[k][:, nb * 512:(nb + 1) * 512],
                                     start=(k == 0), stop=(k == NCH - 1))

            # ---- LN chain ----
            mu_sb = sb.tile([1, SEQ], F32, tag="musb")
            nc.vector.tensor_copy(mu_sb[:, 0:IB], s1a[0:1, :])
            nc.vector.tensor_copy(mu_sb[:, IB:SEQ], s1b[0:1, :])
            mu_bf = sb.tile([1, SEQ], BF16, tag="mubf")
            nc.vector.tensor_copy(mu_bf[:], mu_sb[:])
            musq = sb.tile([1, SEQ], F32, tag="musq")
            nc.vector.tensor_tensor(musq[:], mu_sb[:], mu_sb[:], ALU.mult)
            varr = sb.tile([1, SEQ], F32, tag="varr")
            nc.vector.tensor_tensor(varr[:], s2[0:1, :], musq[:], ALU.subtract)
            lnv = sb.tile([1, SEQ], F32, tag="lnv")
            nc.scalar.activation(lnv[:], varr[:], AF.Ln, bias=eps_t[0:1, :],
                                 scale=1.0)
            rs_row = sb.tile([1, SEQ], F32, tag="rsr")
            nc.scalar.activation(rs_row[:], lnv[:], AF.Exp, bias=0.0, scale=-0.5)
            irs_bf = sb.tile([1, SEQ], BF16, tag="irs")
            nc.scalar.activation(irs_bf[:], lnv[:], AF.Exp, bias=0.0, scale=0.5)
            rs_b = sb.tile([P, SEQ], F32, tag="rsb")
            nc.gpsimd.partition_broadcast(rs_b[:], rs_row[:], channels=P)

            # ---- projections on raw x; LN folded into fix-matmul + drain ----
            qt = sb.tile([P, SEQ], BF16, tag="qt")
            kt = sb.tile([P, SEQ], BF16, tag="kt")
            vt = sb.tile([P, SEQ], BF16, tag="vt")

            def mains(wname, dsts):
                # dsts: list of (psum_tile, col0) covering SEQ
                for k in range(NCH):
                    for nb in range(4):
                        ps, c0 = dsts[nb]
                        nc.tensor.matmul(ps[:, c0:c0 + 512],
                                         aw[wname][:, k, :],
                                         xt[k][:, nb * 512:(nb + 1) * 512],
                                         start=(k == 0), stop=False)

            def fix(iu, iv, dsts):
                # psum += fu (x) mu  [+ fv (x) 1/rs];  iv None skips bias row
                for nb in range(4):
                    ps, c0 = dsts[nb]
                    blk = slice(nb * 512, (nb + 1) * 512)
                    nc.tensor.matmul(ps[:, c0:c0 + 512], f_t[0:1, iu, :],
                                     mu_bf[:, blk],
                                     start=False, stop=(iv is None))
                    if iv is not None:
                        nc.tensor.matmul(ps[:, c0:c0 + 512], f_t[0:1, iv, :],
                                         irs_bf[:, blk],
                                         start=False, stop=True)

            def drain(dsts, out_sb):
                for nb in range(4):
                    ps, c0 = dsts[nb]
                    nc.vector.tensor_tensor(
                        out_sb[:, nb * 512:(nb + 1) * 512], ps[:, c0:c0 + 512],
                        rs_b[:, nb * 512:(nb + 1) * 512], ALU.mult)

            qm = pa.tile([P, SEQ], F32, tag="av")
            q_dst = [(qm, nb * 512) for nb in range(4)]
            mains("aq", q_dst)
            ka = pa.tile([P, IB], F32, tag="sc0")
            kb = pa.tile([P, IB], F32, tag="sc1")
            k_dst = [(ka, 0), (ka, 512), (kb, 0), (kb, 512)]
            mains("ak", k_dst)
            fix(0, 1, q_dst)
            drain(q_dst, qt)
            fix(2, 3, k_dst)
            drain(k_dst, kt)
            vm = pa.tile([P, SEQ], F32, tag="av")
            v_dst = [(vm, nb * 512) for nb in range(4)]
            mains("av", v_dst)
            fix(4, None, v_dst)
            drain(v_dst, vt)

            # ---- V -> [j, d] layout via PE transpose (bf16) ----
            for g in range(4):
                tr = pa.tile([P, 4, P], BF16, tag=f"sc{g % 2}")
                for t in range(4):
                    nc.tensor.transpose(tr[:, t, :],
                                        vt[:, (4 * g + t) * P:(4 * g + t + 1) * P],
                                        ident[:])
                nc.vector.tensor_copy(
                    v_sb[:, 4 * g:4 * g + 4, :]
                        .rearrange("p t (h x) -> p t h x", h=2)[:, :, :, 64:128],
                    tr[:].rearrange("p t (h c) -> p t h c", h=2))

            # ---- attention + out-proj, per i-block ----
            attn = sb.tile([P, SEQ], BF16, tag="attn")
            for ig in range(2):
                i0 = ig * IB
                av_t = pa.tile([P, HPC, IB], F32, tag="av")
                for h in range(2):
                    hs = slice(h * DH, (h + 1) * DH)

                    def qk(jb):
                        st = pa.tile([P, IB], F32, tag=f"sc{jb % 2}")
                        for half in range(2):
                            nc.tensor.matmul(
                                st[:, half * 512:(half + 1) * 512],
                                kt[hs, jb * P:(jb + 1) * P],
                                qt[hs, i0 + half * 512:i0 + (half + 1) * 512],
                                start=True, stop=True,
                                tile_position=(h * DH, 0))
                        return st

                    def expav(jb, st):
                        e = ep.tile([P, IB], BF16, tag=f"e{jb % 4}")
                        nc.scalar.activation(e[:], st[:], AF.Exp, bias=0.0,
                                             scale=1.0)
                        for half in range(2):
                            nc.tensor.matmul(
                                av_t[:, h, half * 512:(half + 1) * 512],
                                v_sb[:, jb, h * P:(h + 1) * P],
                                e[:, half * 512:(half + 1) * 512],
                                start=(jb == 0), stop=(jb == 15))

                    prev = None
                    for jb in range(16):
                        st = qk(jb)
                        if prev is not None:
                            expav(jb - 1, prev)
                        prev = st
                    expav(15, prev)

                    # normalize head h (overlaps the other head / next phase)
                    se = sb.tile([1, IB], F32, tag=f"se{h}")
                    nc.vector.tensor_copy(se[:], av_t[0:1, h, :])
                    rse = sb.tile([1, IB], F32, tag=f"rse{h}")
                    nc.vector.reciprocal(rse[:], se[:])
                    rb = sb.tile([DH, IB], F32, tag=f"rb{h}")
                    nc.gpsimd.partition_broadcast(rb[:], rse[:], channels=DH)
                    nc.vector.tensor_tensor(attn[hs, i0:i0 + IB],
                                            av_t[DH:P, h, :], rb[:], ALU.mult)

                # out-proj for this i-block, in two 512-col rounds
                for r in range(2):
                    op = pa.tile([P, 4, 512], F32, tag="av")
                    c0 = i0 + r * 512
                    for m in range(4):
                        nc.tensor.matmul(op[:, m, :],
                                         wo_t[:, m * P:(m + 1) * P],
                                         attn[:, c0:c0 + 512],
                                         start=True, stop=True)
                    ysb = sb.tile([P, 4, 512], BF16, tag=f"y{ig}{r}")
                    nc.vector.tensor_copy(ysb[:], op[:])
                    for m in range(4):
                        nc.sync.dma_start(yp_d[m * P:(m + 1) * P, c0:c0 + 512],
                                          ysb[:, m, :])

    nc.compile()
    return nc


def kernel(x, Wq, Wk, Wv, Wo, bo, gamma, beta):
    import ml_dtypes
    from concourse import bass_utils

    BF = ml_dtypes.bfloat16
    x = np.asarray(x, np.float32)
    Wq, Wk, Wv, Wo = (np.asarray(w, np.float32) for w in (Wq, Wk, Wv, Wo))
    bo, gamma, beta = (np.asarray(v, np.float32) for v in (bo, gamma, beta))
    b = x.shape[0]
    xs = x.reshape(b, C, SEQ)
    xs_bf = xs.astype(BF)

    s = DH ** -0.5
    aq_f = gamma[:, None] * Wq * s
    ak_f = gamma[:, None] * Wk
    av_f = gamma[:, None] * Wv
    vq_f = (Wq.T @ beta) * s
    vk_f = Wk.T @ beta
    vv_f = Wv.T @ beta

    def wprep(a):  # [C, 128] -> [128, NCH*128] (p k m)
        return np.ascontiguousarray(
            a.reshape(NCH, P, -1).transpose(1, 0, 2).reshape(P, C)).astype(BF)

    if "nc" not in _CACHE:
        _CACHE["nc"] = _build()
    nc = _CACHE["nc"]

    in_maps = []
    for core in range(8):
        bi, hg = divmod(core, 4)
        cs = slice(hg * P, (hg + 1) * P)
        in_maps.append({
            "xb": np.ascontiguousarray(xs_bf[bi]),
            "aq": wprep(aq_f[:, cs]),
            "ak": wprep(ak_f[:, cs]),
            "av": wprep(av_f[:, cs]),
            "wo": np.ascontiguousarray(Wo[cs, :]).astype(BF),
            "f": np.concatenate([-aq_f[:, cs].sum(0), vq_f[cs],
                                 -ak_f[:, cs].sum(0), vk_f[cs],
                                 -av_f[:, cs].sum(0)])[None, :].astype(BF),
        })

    global _LAST_IN_MAPS
    _LAST_IN_MAPS = in_maps
    res = bass_utils.run_bass_kernel_spmd(nc, in_maps, core_ids=list(range(8)))
    bias_total = bo + Wo.T @ vv_f
    y = np.empty((b, C, SEQ), np.float32)
    for bi in range(b):
        acc = xs[bi] + bias_total[:, None]
        for hg in range(4):
            acc = acc + res.results[bi * 4 + hg]["yp"].astype(np.float32)
        y[bi] = acc
    return y.reshape(x.shape).astype(np.float32)


# revision 12
# speedup vs baseline: 1.8757x; 1.1227x over previous
"""Trainium2 Bass kernel for nn_CrossAttentionBlock (LN -> MHA -> out-proj -> residual).

Sharding: 8 cores = 2 batches x 4 head-groups (2 heads each). Each core:
  - streams raw x (bf16), computes LN stats via ones-matmul while projecting
    Q/K/V on raw x; the LN scale rs and mean/bias corrections are folded into
    the PSUM drain (elementwise *rs) and a rank-2 fix matmul,
  - software-pipelined attention: QK -> exp(ACT) -> AV with double-buffered
    score banks so PE and ACT both stay busy,
  - sumexp via ones-column embedded in the V weights; per-head normalize
    overlaps the other head's attention,
  - out-proj partial written as bf16; host sums the 4 partials per batch and
    adds bias + residual.
"""
import numpy as np

C = 512
SEQ = 2048
P = 128
NCH = 4          # c chunks of 128
DH = 64
HPC = 2          # heads per core
IB = 1024        # attention i-block
EPS = 1e-5

_CACHE = {}
_LAST_IN_MAPS = None


def _build():
    import concourse.bass as bass
    import concourse.tile as tile
    from concourse import bacc, mybir
    from concourse.masks import make_identity

    F32 = mybir.dt.float32
    BF16 = mybir.dt.bfloat16
    AF = mybir.ActivationFunctionType
    ALU = mybir.AluOpType

    nc = bacc.Bacc("TRN2", target_bir_lowering=False, debug=False,
                   enable_asserts=False, num_devices=8)

    xb_d = nc.dram_tensor("xb", [C, SEQ], BF16, kind="ExternalInput").ap()
    aq_d = nc.dram_tensor("aq", [P, C], BF16, kind="ExternalInput").ap()
    ak_d = nc.dram_tensor("ak", [P, C], BF16, kind="ExternalInput").ap()
    av_d = nc.dram_tensor("av", [P, C], BF16, kind="ExternalInput").ap()
    wo_d = nc.dram_tensor("wo", [P, C], BF16, kind="ExternalInput").ap()
    # rank-1 fix weight rows packed on one partition: [qu, qv, ku, kv, vu]
    f_d = nc.dram_tensor("f", [1, 5 * P], BF16, kind="ExternalInput").ap()
    yp_d = nc.dram_tensor("yp", [C, SEQ], BF16, kind="ExternalOutput").ap()

    with tile.TileContext(nc) as tc:
        with tc.tile_pool(name="sb", bufs=1) as sb, \
             tc.tile_pool(name="ep", bufs=1) as ep, \
             tc.tile_pool(name="pa", bufs=1, space="PSUM") as pa:

            # ---- input DMAs (x first; stats start as chunks land) ----
            xt = []
            for k in range(NCH):
                t = sb.tile([P, SEQ], BF16, tag=f"x{k}")
                nc.sync.dma_start(t[:], xb_d[k * P:(k + 1) * P, :])
                xt.append(t)
            aw = {}
            for name, d in (("aq", aq_d), ("ak", ak_d), ("av", av_d)):
                t = sb.tile([P, NCH, P], BF16, tag=name)
                nc.sync.dma_start(t[:], d.rearrange("p (k m) -> p k m", k=NCH))
                aw[name] = t
            wo_t = sb.tile([P, C], BF16, tag="wo")
            nc.sync.dma_start(wo_t[:], wo_d[:, :])
            f_t = sb.tile([1, 5, P], BF16, tag="f")
            nc.sync.dma_start(f_t[:], f_d.rearrange("o (i p) -> o i p", i=5))

            # ---- constants ----
            ones_t = sb.tile([P, 2], BF16, tag="ones")
            nc.vector.memset(ones_t[:], 1.0 / C)
            eps_t = sb.tile([1, 1], F32, tag="eps")
            nc.vector.memset(eps_t[:], EPS)
            ident_f = sb.tile([P, P], F32, tag="idf")
            make_identity(nc, ident_f[:])
            ident = sb.tile([P, P], BF16, tag="id")
            nc.vector.tensor_copy(ident[:], ident_f[:])
            v_sb = sb.tile([P, 16, 256], BF16, tag="vsb")
            nc.vector.memset(v_sb[:], 0.0)
            nc.vector.memset(
                v_sb[:].rearrange("p j (h c) -> p j h c", c=128)[:, :, :, 0:1], 1.0)

            # ---- squares (DVE, per chunk as x lands) ----
            xq = []
            for k in range(NCH):
                t = sb.tile([P, SEQ], BF16, tag=f"q{k}")
                nc.vector.tensor_tensor(t[:], xt[k][:], xt[k][:], ALU.mult)
                xq.append(t)

            # ---- LN stats: s1 (mean) in sc0/sc1, s2 (E[x^2]) in av ----
            s1a = pa.tile([2, IB], F32, tag="sc0")
            s1b = pa.tile([2, IB], F32, tag="sc1")
            for k in range(NCH):
                for nb in range(4):
                    dst = s1a if nb < 2 else s1b
                    nc.tensor.matmul(dst[:, (nb % 2) * 512:(nb % 2 + 1) * 512],
                                     ones_t[:],
                                     xt[k][:, nb * 512:(nb + 1) * 512],
                                     start=(k == 0), stop=(k == NCH - 1))
            s2 = pa.tile([2, SEQ], F32, tag="av")
            for k in range(NCH):
                for nb in range(4):
                    nc.tensor.matmul(s2[:, nb * 512:(nb + 1) * 512],
                                     ones_t[:],
                                     xq[k][:, nb * 512:(nb + 1) * 512],
                                     start=(k == 0), stop=(k == NCH - 1))

            # ---- LN chain ----
            mu_sb = sb.tile([1, SEQ], F32, tag="musb")
            nc.vector.tensor_copy(mu_sb[:, 0:IB], s1a[0:1, :])
            nc.vector.tensor_copy(mu_sb[:, IB:SEQ], s1b[0:1, :])
            mu_bf = sb.tile([1, SEQ], BF16, tag="mubf")
            nc.vector.tensor_copy(mu_bf[:], mu_sb[:])
            musq = sb.tile([1, SEQ], F32, tag="musq")
            nc.vector.tensor_tensor(musq[:], mu_sb[:], mu_sb[:], ALU.mult)
            varr = sb.tile([1, SEQ], F32, tag="varr")
            nc.vector.tensor_tensor(varr[:], s2[0:1, :], musq[:], ALU.subtract)
            lnv = sb.tile([1, SEQ], F32, tag="lnv")
            nc.scalar.activation(lnv[:], varr[:], AF.Ln, bias=eps_t[0:1, :],
                                 scale=1.0)
            rs_row = sb.tile([1, SEQ], F32, tag="rsr")
            nc.scalar.activation(rs_row[:], lnv[:], AF.Exp, bias=0.0, scale=-0.5)
            irs_bf = sb.tile([1, SEQ], BF16, tag="irs")
            nc.scalar.activation(irs_bf[:], lnv[:], AF.Exp, bias=0.0, scale=0.5)
            rs_b = sb.tile([P, SEQ], F32, tag="rsb")
            nc.gpsimd.partition_broadcast(rs_b[:], rs_row[:], channels=P)

            # ---- projections on raw x; LN folded into fix-matmul + drain ----
            qt = sb.tile([P, SEQ], BF16, tag="qt")
            kt = sb.tile([P, SEQ], BF16, tag="kt")
            vt = sb.tile([P, SEQ], BF16, tag="vt")

            def mains(wname, dsts):
                # dsts: list of (psum_tile, col0) covering SEQ
                for k in range(NCH):
                    for nb in range(4):
                        ps, c0 = dsts[nb]
                        nc.tensor.matmul(ps[:, c0:c0 + 512],
                                         aw[wname][:, k, :],
                                         xt[k][:, nb * 512:(nb + 1) * 512],
                                         start=(k == 0), stop=False)

            def fix(iu, iv, dsts):
                # psum += fu (x) mu  [+ fv (x) 1/rs];  iv None skips bias row
                for nb in range(4):
                    ps, c0 = dsts[nb]
                    blk = slice(nb * 512, (nb + 1) * 512)
                    nc.tensor.matmul(ps[:, c0:c0 + 512], f_t[0:1, iu, :],
                                     mu_bf[:, blk],
                                     start=False, stop=(iv is None))
                    if iv is not None:
                        nc.tensor.matmul(ps[:, c0:c0 + 512], f_t[0:1, iv, :],
                                         irs_bf[:, blk],
                                         start=False, stop=True)

            def drain(dsts, out_sb):
                for nb in range(4):
                    ps, c0 = dsts[nb]
                    nc.vector.tensor_tensor(
                        out_sb[:, nb * 512:(nb + 1) * 512], ps[:, c0:c0 + 512],
                        rs_b[:, nb * 512:(nb + 1) * 512], ALU.mult)

            qm = pa.tile([P, SEQ], F32, tag="av")
            q_dst = [(qm, nb * 512) for nb in range(4)]
            mains("aq", q_dst)
            ka = pa.tile([P, IB], F32, tag="sc0")
            kb = pa.tile([P, IB], F32, tag="sc1")
            k_dst = [(ka, 0), (ka, 512), (kb, 0), (kb, 512)]
            mains("ak", k_dst)
            fix(0, 1, q_dst)
            drain(q_dst, qt)
            fix(2, 3, k_dst)
            drain(k_dst, kt)
            vm = pa.tile([P, SEQ], F32, tag="av")
            v_dst = [(vm, nb * 512) for nb in range(4)]
            mains("av", v_dst)
            fix(4, None, v_dst)
            drain(v_dst, vt)

            # ---- V -> [j, d] layout via PE transpose (bf16) ----
            for g in range(4):
                tr = pa.tile([P, 4, P], BF16, tag=f"sc{g % 2}")
                for t in range(4):
                    nc.tensor.transpose(tr[:, t, :],
                                        vt[:, (4 * g + t) * P:(4 * g + t + 1) * P],
                                        ident[:])
                nc.vector.tensor_copy(
                    v_sb[:, 4 * g:4 * g + 4, :]
                        .rearrange("p t (h x) -> p t h x", h=2)[:, :, :, 64:128],
                    tr[:].rearrange("p t (h c) -> p t h c", h=2))

            # ---- attention + out-proj, per i-block ----
            attn = sb.tile([P, SEQ], BF16, tag="attn")
            for ig in range(2):
                i0 = ig * IB
                av_t = pa.tile([P, HPC, IB], F32, tag="av")
                for h in range(2):
                    hs = slice(h * DH, (h + 1) * DH)

                    def qk(jb):
                        st = pa.tile([P, IB], F32, tag=f"sc{jb % 2}")
                        for half in range(2):
                            nc.tensor.matmul(
                                st[:, half * 512:(half + 1) * 512],
                                kt[hs, jb * P:(jb + 1) * P],
                                qt[hs, i0 + half * 512:i0 + (half + 1) * 512],
                                start=True, stop=True,
                                tile_position=(h * DH, 0))
                        return st

                    def expav(jb, st):
                        e = ep.tile([P, IB], BF16, tag=f"e{jb % 4}")
                        nc.scalar.activation(e[:], st[:], AF.Exp, bias=0.0,
                                             scale=1.0)
                        for half in range(2):
                            nc.tensor.matmul(
                                av_t[:, h, half * 512:(half + 1) * 512],
                                v_sb[:, jb, h * P:(h + 1) * P],
                                e[:, half * 512:(half + 1) * 512],
                                start=(jb == 0), stop=(jb == 15))

                    prev = None
                    for jb in range(16):
                        st = qk(jb)
                        if prev is not None:
                            expav(jb - 1, prev)
                        prev = st
                    expav(15, prev)

                    # normalize head h (overlaps the other head / next phase)
                    se = sb.tile([1, IB], F32, tag=f"se{h}")
                    nc.vector.tensor_copy(se[:], av_t[0:1, h, :])
                    rse = sb.tile([1, IB], F32, tag=f"rse{h}")
                    nc.vector.reciprocal_approx_fast(rse[:], se[:])
                    rb = sb.tile([DH, IB], F32, tag=f"rb{h}")
                    nc.gpsimd.partition_broadcast(rb[:], rse[:], channels=DH)
                    nc.vector.tensor_tensor(attn[hs, i0:i0 + IB],
                                            av_t[DH:P, h, :], rb[:], ALU.mult)

                # out-proj for this i-block, in two 512-col rounds
                for r in range(2):
                    op = pa.tile([P, 4, 512], F32, tag="av")
                    c0 = i0 + r * 512
                    for m in range(4):
                        nc.tensor.matmul(op[:, m, :],
                                         wo_t[:, m * P:(m + 1) * P],
                                         attn[:, c0:c0 + 512],
                                         start=True, stop=True)
                    ysb = sb.tile([P, 4, 512], BF16, tag=f"y{ig}{r}")
                    nc.vector.tensor_copy(ysb[:], op[:])
                    nc.sync.dma_start(
                        yp_d[:, c0:c0 + 512].rearrange("(m p) n -> p m n", p=P),
                        ysb[:])

    nc.compile()
    return nc


def kernel(x, Wq, Wk, Wv, Wo, bo, gamma, beta):
    import ml_dtypes
    from concourse import bass_utils

    BF = ml_dtypes.bfloat16
    x = np.asarray(x, np.float32)
    Wq, Wk, Wv, Wo = (np.asarray(w, np.float32) for w in (Wq, Wk, Wv, Wo))
    bo, gamma, beta = (np.asarray(v, np.float32) for v in (bo, gamma, beta))
    b = x.shape[0]
    xs = x.reshape(b, C, SEQ)
    xs_bf = xs.astype(BF)

    s = DH ** -0.5
    aq_f = gamma[:, None] * Wq * s
    ak_f = gamma[:, None] * Wk
    av_f = gamma[:, None] * Wv
    vq_f = (Wq.T @ beta) * s
    vk_f = Wk.T @ beta
    vv_f = Wv.T @ beta

    def wprep(a):  # [C, 128] -> [128, NCH*128] (p k m)
        return np.ascontiguousarray(
            a.reshape(NCH, P, -1).transpose(1, 0, 2).reshape(P, C)).astype(BF)

    if "nc" not in _CACHE:
        _CACHE["nc"] = _build()
    nc = _CACHE["nc"]

    in_maps = []
    for core in range(8):
        bi, hg = divmod(core, 4)
        cs = slice(hg * P, (hg + 1) * P)
        in_maps.append({
            "xb": np.ascontiguousarray(xs_bf[bi]),
            "aq": wprep(aq_f[:, cs]),
            "ak": wprep(ak_f[:, cs]),
            "av": wprep(av_f[:, cs]),
            "wo": np.ascontiguousarray(Wo[cs, :]).astype(BF),
            "f": np.concatenate([-aq_f[:, cs].sum(0), vq_f[cs],
                                 -ak_f[:, cs].sum(0), vk_f[cs],
                                 -av_f[:, cs].sum(0)])[None, :].astype(BF),
        })

    global _LAST_IN_MAPS
    _LAST_IN_MAPS = in_maps
    res = bass_utils.run_bass_kernel_spmd(nc, in_maps, core_ids=list(range(8)))
    bias_total = bo + Wo.T @ vv_f
    y = np.empty((b, C, SEQ), np.float32)
    for bi in range(b):
        acc = xs[bi] + bias_total[:, None]
        for hg in range(4):
            acc = acc + res.results[bi * 4 + hg]["yp"].astype(np.float32)
        y[bi] = acc
    return y.reshape(x.shape).astype(np.float32)
